# revision 4
# baseline (speedup 1.0000x reference)
"""2-layer GAT on 8 Trainium2 NeuronCores.

Strategy (sharding_hint: partition nodes + incident edges, replicate params):
- Destination nodes are sharded across the 8 cores (12544-node aligned slabs).
- The host performs the halo exchange / edge gather: for every edge (s -> d)
  owned by core(d), it stages x[s] (layer 1) / h1[s] (layer 2) into a
  dst-sorted, 128-padded slot array.  All heavy math runs on-device:
  per-edge feature transform (PE matmul vs W), leaky-relu + exp of attention
  scores (ACT), attention-weighted segment-sum via one-hot scatter matmuls
  (PE), softmax normalization, bias, relu / log_softmax (DVE/ACT).
- Layer 2 needs layer-1 output for gathering, so the kernel runs as two
  SPMD launches with a host halo exchange in between.

Edge slot layout per core: 98 dst tiles x BPD blocks x 128 edges.
Slot e = b*128 + p lives at [p, b] of every [128, NB] array; block b belongs
to dst tile b // BPD.  Padding slots carry z = -1e9 so exp(z) == 0 exactly.

Softmax uses raw exp (no per-segment max): scores are bounded (|z| < ~20),
and softmax is shift-invariant, so this matches the reference numerically.
"""

import numpy as np
import ml_dtypes

import concourse.bacc as bacc
import concourse.mybir as mybir
import concourse.tile as tile
from concourse.bass_utils import run_bass_kernel_spmd

P = 128
NCORES = 8
N_NODES = 100000
F_IN, F_HID, F_OUT = 128, 64, 40
DPC = 12544           # dst nodes per core (98 tiles of 128)
TPD = 98              # dst tiles per core
NEG_SLOPE = 0.2

_BF16 = ml_dtypes.bfloat16
_COMPILED = {}


# ---------------------------------------------------------------- device ----

def _build_layer(fi, fout, bpd, tpd, final):
    """One GAT layer as an SPMD Bass kernel.

    Inputs per core:
      xgt  [fi, NB*128] bf16 : column e = (input features of src of edge e)^T
      z    [128, NB]  f32    : per-edge raw attention score (pads: -1e9)
      dl   [128, NB]  int16  : dst index within its 128-dst tile
      wb   [fi, fout] bf16   : weight matrix
      brep [128, fout] f32   : bias replicated across partitions
    Output: out [tpd*128, fout] f32 (rows = this core's dst nodes)
    """
    nb = tpd * bpd
    kd = max(1, 48 // bpd)          # dst tiles per stream chunk
    ch = kd * bpd                   # blocks per chunk
    grp = 512 // fout               # expand-psum blocks per PSUM bank

    nc = bacc.Bacc("TRN2", num_devices=NCORES)
    xgt_d = nc.declare_dram_parameter("xgt", [fi, nb * 128], mybir.dt.bfloat16, isOutput=False)
    z_d = nc.declare_dram_parameter("z", [P, nb], mybir.dt.float32, isOutput=False)
    dl_d = nc.declare_dram_parameter("dl", [P, nb], mybir.dt.int16, isOutput=False)
    wb_d = nc.declare_dram_parameter("wb", [fi, fout], mybir.dt.bfloat16, isOutput=False)
    br_d = nc.declare_dram_parameter("brep", [P, fout], mybir.dt.float32, isOutput=False)
    out_d = nc.declare_dram_parameter("out", [tpd * 128, fout], mybir.dt.float32, isOutput=True)

    with tile.TileContext(nc) as tc:
        with (
            tc.tile_pool(name="const", bufs=1) as cpool,
            tc.tile_pool(name="sb", bufs=2) as pool,
            tc.tile_pool(name="fin", bufs=3) as fpool,
            tc.tile_pool(name="pe", bufs=2, space="PSUM") as pepool,
            tc.tile_pool(name="psd", bufs=3, space="PSUM") as sdpool,
        ):
            wb_t = cpool.tile([fi, fout], mybir.dt.bfloat16)
            nc.sync.dma_start(out=wb_t[:], in_=wb_d[:])
            br_t = cpool.tile([P, fout], mybir.dt.float32)
            nc.sync.dma_start(out=br_t[:], in_=br_d[:])
            iota_t = cpool.tile([P, P], mybir.dt.int16)
            nc.gpsimd.iota(iota_t[:], pattern=[[1, P]], base=0, channel_multiplier=0)
            dl_t = cpool.tile([P, nb], mybir.dt.int16)
            nc.sync.dma_start(out=dl_t[:], in_=dl_d[:])
            z_t = cpool.tile([P, nb], mybir.dt.float32)
            nc.sync.dma_start(out=z_t[:], in_=z_d[:])
            ost = cpool.tile([P, tpd, fout], mybir.dt.float32)

            for c0 in range(0, nb, ch):
                cn = min(ch, nb - c0)
                xg_c = pool.tile([fi, ch * P], mybir.dt.bfloat16, tag="xg")
                nc.sync.dma_start(out=xg_c[:, : cn * P], in_=xgt_d[:, c0 * P:(c0 + cn) * P])

                oh_c = pool.tile([P, ch, P], mybir.dt.bfloat16, tag="oh")
                nc.vector.tensor_tensor(
                    out=oh_c[:, :cn, :],
                    in0=dl_t[:, c0:c0 + cn].unsqueeze(2).to_broadcast([P, cn, P]),
                    in1=iota_t[:].unsqueeze(1).to_broadcast([P, cn, P]),
                    op=mybir.AluOpType.is_equal,
                )

                lr_c = pool.tile([P, ch], mybir.dt.float32, tag="lr")
                # leaky_relu(z) = max(z, 0.2*z), exact on DVE
                nc.vector.scalar_tensor_tensor(
                    out=lr_c[:, :cn], in0=z_t[:, c0:c0 + cn], scalar=NEG_SLOPE,
                    in1=z_t[:, c0:c0 + cn],
                    op0=mybir.AluOpType.mult, op1=mybir.AluOpType.max,
                )
                ex_c = pool.tile([P, ch], mybir.dt.float32, tag="ex")
                nc.scalar.activation(out=ex_c[:, :cn], in_=lr_c[:, :cn],
                                     func=mybir.ActivationFunctionType.Exp)

                vals_c = pool.tile([P, ch, fout + 1], mybir.dt.bfloat16, tag="vals")
                nc.vector.tensor_copy(out=vals_c[:, :cn, fout], in_=ex_c[:, :cn])

                # per-edge transformed features: xg.T @ W, scaled by ex
                for g0 in range(0, cn, grp):
                    gn = min(grp, cn - g0)
                    peps = pepool.tile([P, grp * fout], mybir.dt.float32, tag="pe")
                    for j in range(gn):
                        b = g0 + j
                        nc.tensor.matmul(
                            out=peps[:, j * fout:(j + 1) * fout],
                            lhsT=xg_c[:, b * P:(b + 1) * P],
                            rhs=wb_t[:],
                            start=True, stop=True,
                        )
                    nc.vector.tensor_tensor(
                        out=vals_c[:, g0:g0 + gn, 0:fout],
                        in0=peps[:, : gn * fout].rearrange("p (g f) -> p g f", f=fout),
                        in1=ex_c[:, g0:g0 + gn].unsqueeze(2).to_broadcast([P, gn, fout]),
                        op=mybir.AluOpType.mult,
                    )

                # scatter into dst tiles (kd dtiles per chunk, bpd blocks each)
                for td in range(kd):
                    t = c0 // bpd + td
                    if t >= tpd:
                        break
                    psd = sdpool.tile([P, fout + 1], mybir.dt.float32, tag="psd")
                    for j in range(bpd):
                        b = td * bpd + j
                        nc.tensor.matmul(
                            out=psd[:],
                            lhsT=oh_c[:, b, :],
                            rhs=vals_c[:, b, :],
                            start=(j == 0), stop=(j == bpd - 1),
                        )
                    # finalize dst tile t
                    rc = fpool.tile([P, 1], mybir.dt.float32, tag="rc")
                    nc.vector.reciprocal(out=rc[:], in_=psd[:, fout:fout + 1])
                    ot = fpool.tile([P, fout], mybir.dt.float32, tag="ot")
                    nc.vector.scalar_tensor_tensor(
                        out=ot[:], in0=psd[:, 0:fout], scalar=rc[:], in1=br_t[:],
                        op0=mybir.AluOpType.mult, op1=mybir.AluOpType.add,
                    )
                    if final == "relu":
                        nc.scalar.activation(out=ost[:, t, :], in_=ot[:],
                                             func=mybir.ActivationFunctionType.Relu)
                    else:  # log_softmax
                        mx = fpool.tile([P, 1], mybir.dt.float32, tag="mx")
                        nc.vector.tensor_reduce(out=mx[:], in_=ot[:],
                                                axis=mybir.AxisListType.X,
                                                op=mybir.AluOpType.max)
                        sh = fpool.tile([P, fout], mybir.dt.float32, tag="sh")
                        nc.vector.tensor_scalar_sub(sh[:], ot[:], mx[:])
                        es = fpool.tile([P, fout], mybir.dt.float32, tag="es")
                        se = fpool.tile([P, 1], mybir.dt.float32, tag="se")
                        nc.scalar.activation(out=es[:], in_=sh[:],
                                             func=mybir.ActivationFunctionType.Exp,
                                             accum_out=se[:])
                        lse = fpool.tile([P, 1], mybir.dt.float32, tag="lse")
                        nc.scalar.activation(out=lse[:], in_=se[:],
                                             func=mybir.ActivationFunctionType.Ln)
                        nc.vector.tensor_scalar_sub(ost[:, t, :], sh[:], lse[:])

            nc.sync.dma_start(
                out=out_d[:].rearrange("(t p) f -> p t f", p=P),
                in_=ost[:],
            )
    nc.compile()
    return nc


def _layer_nc(fi, fout, bpd, tpd, final):
    key = (fi, fout, bpd, tpd, final)
    if key not in _COMPILED:
        _COMPILED[key] = _build_layer(fi, fout, bpd, tpd, final)
    return _COMPILED[key]


# ------------------------------------------------------------------ host ----

def _edge_slots(edge_index):
    """Sort edges per owning core into dst-tile slots.  Returns per-core
    (slot_src int64 [NSLOT] with -1 pads, dl int16 [128, NB]) + BPD."""
    n_nodes, dpc, tpd, ncores = N_NODES, DPC, TPD, NCORES
    loops = np.arange(n_nodes, dtype=np.int64)
    src = np.concatenate([np.asarray(edge_index[0], np.int64), loops])
    dst = np.concatenate([np.asarray(edge_index[1], np.int64), loops])
    core = np.minimum(dst // dpc, ncores - 1)

    percore = []
    bpd = 1
    for k in range(ncores):
        m = core == k
        s_k, d_k = src[m], dst[m]
        dloc = d_k - dpc * k
        t = dloc // P
        order = np.argsort(t, kind="stable")
        s_k, d_k, dloc, t = s_k[order], d_k[order], dloc[order], t[order]
        counts = np.bincount(t, minlength=tpd)
        bpd = max(bpd, int(np.ceil(counts.max() / P)))
        percore.append((s_k, dloc, t, counts))

    out = []
    nslot = tpd * bpd * P
    for (s_k, dloc, t, counts) in percore:
        starts = np.zeros(tpd, np.int64)
        starts[1:] = np.cumsum(counts)[:-1]
        rank = np.arange(len(t)) - starts[t]
        slot = t * (bpd * P) + rank
        slot_src = np.full(nslot, -1, np.int64)
        slot_src[slot] = s_k
        dl = np.zeros(nslot, np.int16)
        dl[slot] = (dloc % P).astype(np.int16)
        nb = tpd * bpd
        out.append((slot_src, dl.reshape(nb, P).T.copy()))
    return out, bpd


def _slot_f32(vals_per_edge, slot_src_valid_mask, slot_fill, nb):
    a = np.full(nb * P, slot_fill, np.float32)
    a[slot_src_valid_mask] = vals_per_edge
    return a.reshape(nb, P).T.copy()


def _stage_layer(slots, bpd, xfeat, w, a_s, a_d):
    """Per-core staged inputs for one GAT layer."""
    nb = TPD * bpd
    xb = np.ascontiguousarray(xfeat.astype(_BF16))
    ws = (w @ a_s).astype(np.float32)
    wd = (w @ a_d).astype(np.float32)
    als = (xfeat @ ws).astype(np.float32)
    ald = (xfeat @ wd).astype(np.float32)
    fi = xfeat.shape[1]
    maps = []
    for k, (slot_src, dl) in enumerate(slots):
        m = slot_src >= 0
        sgt = np.zeros((fi, nb * P), _BF16)
        sgt[:, m] = xb[slot_src[m]].T
        # z score per edge; dst of slot e: tile t = (e//128)//bpd, local = dl
        e = np.arange(nb * P)
        dglob = DPC * k + (e // (bpd * P)) * P + dl.T.reshape(-1)
        z = np.full(nb * P, -1e9, np.float32)
        zm = als[slot_src[m]] + ald[np.minimum(dglob[m], N_NODES - 1)]
        z[m] = zm
        maps.append({
            "xgt": sgt,
            "z": z.reshape(nb, P).T.copy(),
            "dl": dl,
        })
    return maps


def _run_layer(maps, w, b, fi, fout, bpd, final):
    nc = _layer_nc(fi, fout, bpd, TPD, final)
    wb = np.ascontiguousarray(w.astype(_BF16))
    brep = np.tile(np.asarray(b, np.float32)[None, :], (P, 1))
    in_maps = [{**m, "wb": wb, "brep": brep} for m in maps]
    res = run_bass_kernel_spmd(nc, in_maps, list(range(NCORES))).results
    return np.concatenate([res[k]["out"] for k in range(NCORES)], axis=0)


# ---------------------------------------------------------------- kernel ----

def kernel(x, edge_index, W1, a_src1, a_dst1, b1, W2, a_src2, a_dst2, b2):
    x = np.asarray(x, np.float32)
    edge_index = np.asarray(edge_index)
    W1, a_src1, a_dst1, b1 = (np.asarray(a, np.float32) for a in (W1, a_src1, a_dst1, b1))
    W2, a_src2, a_dst2, b2 = (np.asarray(a, np.float32) for a in (W2, a_src2, a_dst2, b2))

    slots, bpd = _edge_slots(edge_index)

    maps1 = _stage_layer(slots, bpd, x, W1, a_src1, a_dst1)
    h1 = _run_layer(maps1, W1, b1, F_IN, F_HID, bpd, "relu")[:N_NODES]

    maps2 = _stage_layer(slots, bpd, h1, W2, a_src2, a_dst2)
    out = _run_layer(maps2, W2, b2, F_HID, F_OUT, bpd, "logsoftmax")[:N_NODES]
    return np.ascontiguousarray(out, dtype=np.float32)
